# revision 18
# baseline (speedup 1.0000x reference)
"""GNN message-passing kernel for Trainium2 (8 NeuronCores, edge-data-parallel).

Math (reference):
    eq = einsum('dnf,fr->ndr', Xs, Wq)                  # [N, D, R]
    ek = einsum('dnf,dfr->ndr', Xs, Wk)                 # [N, D, R]
    w_ij = sum_d eq[n_i]*ek[n_j]                        # [E, R]
    out  = mlp_w(w_ij) * mlp_t(t_ij)                    # [E, F]

Strategy (per the sharding hint: "shard edges across devices; replicate
the small node-level eq/ek tensors and all weights on every device"):
  - The host computes the node-level eq/ek projection tables once (f32).
    Per-edge DMA gather on TRN2 runs at ~1-3us/index (serial descriptor
    processing in the Q7 ucode -- unusable), so the per-edge table
    gather happens on the host.  The degree-dot (sum_d eq*ek -> R=64, a
    0.6% sliver of total FLOPs) is fused into that gather, so the
    per-edge stream shipped to the device is w_ij bf16 -- 128 B/edge
    instead of the 1 KiB/edge of separate eq/ek streams.  All MLP
    matmul FLOPs (99.4% of the total) run on device.
  - The w stream packs subtile PAIRS into the 128 partitions (DMA
    engines stripe by partition/16, so a 64-partition transfer runs at
    half rate); stage-1 of mlp_w contracts rows [0,64) or [64,128)
    against a correspondingly duplicated w1.
  - Device per 500-edge subtile: 4 matmuls (mm1w 64-contract, mm1t,
    mm2w, mm2t) into a single 2-bank PSUM arena reused for stage-1 then
    stage-2 (their lifetimes don't overlap, so bufs=4 keeps 4 subtiles
    in flight in 8 banks); ONE merged 1000-col Act relu covers both
    branches' stage-1; p2w is staged to SBUF (DVE has one PSUM read
    port, so a two-PSUM-operand tensor_tensor is illegal -- the copy
    alternates DVE/Act to balance their load) and the final product
    runs on DVE, landing in a bf16 [128f, e] slab.
  - DMA: w+tt slabs on the SP queue (first phase chunked to cut
    startup), out quarter-slabs + packed weights on the Pool queue.
"""

import sys

if "/opt/trn_rl_repo" not in sys.path:
    sys.path.insert(0, "/opt/trn_rl_repo")

import ml_dtypes
import numpy as np

BF16 = ml_dtypes.bfloat16

# Problem dims (hardcoded per spec nn_HTR_7464653160731)
D, N, F, R, E, H = 4, 20000, 128, 64, 320000, 128
NCORES = 8

SUB = 500                 # edge subtile (matmul/PSUM granularity)
PHE = 10000               # edges per stream slab (one w/tt DMA each)
NSUBP = PHE // SUB        # subtiles per slab
EPAD = ((E // NCORES) + PHE - 1) // PHE * PHE   # 40000: no padding

_prog_cache = {}


def _build_program(epad, b2t_nonzero, b2w_nonzero, repeat=1,
                   b1t_nonzero=False, b1w_nonzero=False):
    import concourse.bacc as bacc
    import concourse.mybir as mybir
    import concourse.tile as tile

    f32 = mybir.dt.float32
    bf16 = mybir.dt.bfloat16
    AF = mybir.ActivationFunctionType
    ALU = mybir.AluOpType
    AX = mybir.AxisListType

    n_ph = epad // PHE
    nsp2 = NSUBP // 2
    b2_any = b2t_nonzero or b2w_nonzero

    nc = bacc.Bacc("TRN2", target_bir_lowering=False)

    # DRAM I/O.  wt holds per-edge w_ij rows packed two subtiles deep:
    # partition a*64+r of column (ph, u, c) is w_ij[edge (ph, 2u+a, c), r].
    wt = nc.dram_tensor("wt", [128, n_ph, nsp2, SUB], bf16,
                        kind="ExternalInput")
    tt = nc.dram_tensor("tt", [128, epad], bf16, kind="ExternalInput")
    mw1 = nc.dram_tensor("mw1", [128, H], bf16, kind="ExternalInput")
    mw2 = nc.dram_tensor("mw2", [128, F], bf16, kind="ExternalInput")
    mt1 = nc.dram_tensor("mt1", [128, H], bf16, kind="ExternalInput")
    mt2 = nc.dram_tensor("mt2", [128, F], bf16, kind="ExternalInput")
    bias = nc.dram_tensor("bias", [128, 4], f32, kind="ExternalInput")
    out = nc.dram_tensor("out", [128, epad], bf16, kind="ExternalOutput")

    any_bias = b1w_nonzero or b1t_nonzero or b2_any

    with tile.TileContext(nc) as tc:
        with tc.tile_pool(name="const", bufs=1) as cpool:
            mw1_sb = cpool.tile([128, H], bf16)
            mw2_sb = cpool.tile([128, F], bf16)
            mt1_sb = cpool.tile([128, H], bf16)
            mt2_sb = cpool.tile([128, F], bf16)
            consts = [(mw1_sb, mw1), (mw2_sb, mw2),
                      (mt1_sb, mt1), (mt2_sb, mt2)]
            if any_bias:
                bias_sb = cpool.tile([128, 4], f32)
                consts.append((bias_sb, bias))
            for sb_t, dr in consts:
                nc.gpsimd.dma_start(sb_t[:], dr[:])

            for _rep in range(repeat):
                with (
                    tc.tile_pool(name="wsl", bufs=2) as wsl,
                    tc.tile_pool(name="tsl", bufs=2) as tsl,
                    tc.tile_pool(name="osl", bufs=2) as osl,
                    tc.tile_pool(name="hbuf", bufs=6) as hbuf,
                    tc.tile_pool(name="swb", bufs=6) as swb,
                    tc.tile_pool(name="psm", bufs=4, space="PSUM") as psm,
                ):
                    for ph in range(n_ph):
                        sl_ph = slice(ph * PHE, (ph + 1) * PHE)
                        w_s = wsl.tile([128, nsp2, SUB], bf16, tag="wt")
                        tt_s = tsl.tile([128, PHE], bf16, tag="tt")
                        ot_s = osl.tile([128, PHE], bf16, tag="ot")
                        if ph == 0:
                            # chunk the first slab loads (1+3+3+3 pairs) so
                            # subtile 0 starts after a ~1-pair DMA
                            bounds = [0, 1, 4, 7, nsp2]
                            for k in range(4):
                                b0, b1 = bounds[k], bounds[k + 1]
                                nc.sync.dma_start(
                                    w_s[:, b0:b1, :],
                                    wt[:, ph, b0:b1, :])
                                nc.sync.dma_start(
                                    tt_s[:, b0 * 2 * SUB:b1 * 2 * SUB],
                                    tt[:, ph * PHE + b0 * 2 * SUB:
                                       ph * PHE + b1 * 2 * SUB])
                        else:
                            nc.sync.dma_start(w_s[:], wt[:, ph, :, :])
                            nc.sync.dma_start(tt_s[:], tt[:, sl_ph])
                        for j in range(NSUBP):
                            sl = slice(j * SUB, (j + 1) * SUB)
                            u, a = divmod(j, 2)
                            # one 2-bank PSUM arena per subtile, reused for
                            # stage-1 (p1w|p1t) then stage-2 (p2w|p2t): their
                            # lifetimes don't overlap, so bufs=4 gives 4
                            # subtiles in flight within 8 banks.
                            pq = psm.tile([128, 2, 512], f32, tag="pq")
                            nc.tensor.matmul(pq[:, 0, 0:SUB],
                                             mw1_sb[a * R:(a + 1) * R, :],
                                             w_s[a * R:(a + 1) * R, u, :],
                                             start=True, stop=True)
                            nc.tensor.matmul(pq[:, 1, 0:SUB], mt1_sb[:],
                                             tt_s[:, sl],
                                             start=True, stop=True)
                            h_t = hbuf.tile([128, 2, SUB], bf16, tag="h")
                            if b1w_nonzero or b1t_nonzero:
                                nc.scalar.activation(h_t[:, 0, :],
                                                     pq[:, 0, 0:SUB], AF.Relu,
                                                     bias=bias_sb[:, 0:1])
                                nc.scalar.activation(h_t[:, 1, :],
                                                     pq[:, 1, 0:SUB], AF.Relu,
                                                     bias=bias_sb[:, 1:2])
                            else:
                                # one 2*SUB-col act covers both relus
                                nc.scalar.activation(h_t[:],
                                                     pq[:, :, 0:SUB], AF.Relu)
                            # stage-2 overwrites the arena (WAR via h_t dep)
                            nc.tensor.matmul(pq[:, 0, 0:SUB], mw2_sb[:],
                                             h_t[:, 0, :],
                                             start=True, stop=True)
                            nc.tensor.matmul(pq[:, 1, 0:SUB], mt2_sb[:],
                                             h_t[:, 1, :],
                                             start=True, stop=True)
                            sw_t = swb.tile([128, SUB], f32, tag="sw")
                            if b2w_nonzero:
                                nc.scalar.activation(sw_t[:], pq[:, 0, 0:SUB],
                                                     AF.Identity,
                                                     bias=bias_sb[:, 2:3])
                            elif j % 5 == 2:
                                # ~20% of staging copies on Act to balance
                                nc.scalar.activation(sw_t[:], pq[:, 0, 0:SUB],
                                                     AF.Identity)
                            else:
                                nc.vector.tensor_copy(sw_t[:], pq[:, 0, 0:SUB])
                            if b2t_nonzero:
                                st_t = swb.tile([128, SUB], f32, tag="st")
                                nc.scalar.activation(st_t[:], pq[:, 1, 0:SUB],
                                                     AF.Identity,
                                                     bias=bias_sb[:, 3:4])
                                nc.vector.tensor_mul(ot_s[:, sl], st_t[:],
                                                     sw_t[:])
                            else:
                                nc.vector.tensor_mul(ot_s[:, sl],
                                                     pq[:, 1, 0:SUB], sw_t[:])
                            if j % (NSUBP // 4) == NSUBP // 4 - 1:
                                # quarter-slab output drains
                                k = j // (NSUBP // 4)
                                qc = PHE // 4
                                nc.gpsimd.dma_start(
                                    out[:, ph * PHE + k * qc:
                                        ph * PHE + (k + 1) * qc],
                                    ot_s[:, k * qc:(k + 1) * qc])

    nc.compile()
    return nc


def get_program(epad, b2t_nonzero, b2w_nonzero, b1t_nonzero=False,
                b1w_nonzero=False):
    key = (epad, b2t_nonzero, b2w_nonzero, b1t_nonzero, b1w_nonzero)
    if key not in _prog_cache:
        _prog_cache[key] = _build_program(
            epad, b2t_nonzero, b2w_nonzero,
            b1t_nonzero=b1t_nonzero, b1w_nonzero=b1w_nonzero)
    return _prog_cache[key]


def kernel(Xs, t_ij, edge_index, Wq, Wk, mw_w1, mw_b1, mw_w2, mw_b2,
           mt_w1, mt_b1, mt_w2, mt_b2):
    from concourse.bass_utils import run_bass_kernel_spmd

    Xs = np.asarray(Xs, np.float32)
    t_ij = np.asarray(t_ij, np.float32)
    edge_index = np.asarray(edge_index)

    esh = E // NCORES                      # edges per core
    epad = (esh + PHE - 1) // PHE * PHE

    nj = edge_index[0].astype(np.int64)
    ni = edge_index[1].astype(np.int64)

    # Node-level eq/ek tables (the replicated arrays from the sharding
    # hint), computed once in f32 on the host.
    Wq_ = np.asarray(Wq, np.float32)
    Wk_ = np.asarray(Wk, np.float32)
    eq_tab = np.einsum("dnf,fr->ndr", Xs, Wq_).astype(np.float32)
    ek_tab = np.einsum("dnf,dfr->ndr", Xs, Wk_).astype(np.float32)

    b1w_nonzero = bool(np.any(np.asarray(mw_b1) != 0))
    b1t_nonzero = bool(np.any(np.asarray(mt_b1) != 0))
    b2w_nonzero = bool(np.any(np.asarray(mw_b2) != 0))
    b2t_nonzero = bool(np.any(np.asarray(mt_b2) != 0))
    nc = get_program(epad, b2t_nonzero, b2w_nonzero, b1t_nonzero,
                     b1w_nonzero)

    bias_arr = np.zeros((128, 4), np.float32)
    bias_arr[:, 0] = np.asarray(mw_b1, np.float32)
    bias_arr[:, 1] = np.asarray(mt_b1, np.float32)
    bias_arr[:, 2] = np.asarray(mw_b2, np.float32)
    bias_arr[:, 3] = np.asarray(mt_b2, np.float32)

    com = {
        "mw1": np.ascontiguousarray(
            np.vstack([np.asarray(mw_w1)] * 2).astype(BF16)),
        "mw2": np.ascontiguousarray(np.asarray(mw_w2).astype(BF16)),
        "mt1": np.ascontiguousarray(np.asarray(mt_w1).astype(BF16)),
        "mt2": np.ascontiguousarray(np.asarray(mt_w2).astype(BF16)),
        "bias": bias_arr,
    }

    in_maps = []
    for g in range(NCORES):
        s0, s1 = g * esh, (g + 1) * esh
        # Fused host gather+degree-dot: w_ij[e, r] = sum_d eq[ni]*ek[nj].
        wij = np.einsum("edr,edr->er", eq_tab[ni[s0:s1]],
                        ek_tab[nj[s0:s1]])
        wpad = np.zeros((epad, R), np.float32)
        wpad[:esh] = wij
        # Pack pairs of SUB-edge subtiles into the 128 partitions:
        # wt[a*64+r, ph, u, c] = wij[edge (ph, 2u+a, c), r]
        n_ph = epad // PHE
        wv = wpad.reshape(n_ph, NSUBP // 2, 2, SUB, R)
        wv = wv.transpose(2, 4, 0, 1, 3).reshape(128, n_ph, NSUBP // 2, SUB)
        tpad = np.zeros((epad, F), np.float32)
        tpad[:esh] = t_ij[s0:s1]
        in_maps.append({
            **com,
            "wt": np.ascontiguousarray(wv.astype(BF16)),
            "tt": np.ascontiguousarray(tpad.T.astype(BF16)),
        })

    res = run_bass_kernel_spmd(nc, in_maps, list(range(NCORES))).results

    result = np.empty((E, F), np.float32)
    for g in range(NCORES):
        o = np.asarray(res[g]["out"]).astype(np.float32)  # [128, epad]
        result[g * esh:(g + 1) * esh] = o[:, :esh].T
    return result


# revision 21
# speedup vs baseline: 1.0637x; 1.0637x over previous
"""GNN message-passing kernel for Trainium2 (8 NeuronCores, edge-data-parallel).

Math (reference):
    eq = einsum('dnf,fr->ndr', Xs, Wq)                  # [N, D, R]
    ek = einsum('dnf,dfr->ndr', Xs, Wk)                 # [N, D, R]
    w_ij = sum_d eq[n_i]*ek[n_j]                        # [E, R]
    out  = mlp_w(w_ij) * mlp_t(t_ij)                    # [E, F]

Strategy (per the sharding hint: "shard edges across devices; replicate
the small node-level eq/ek tensors and all weights on every device"):
  - The host computes the node-level eq/ek projection tables once (f32).
    Per-edge DMA gather on TRN2 runs at ~1-3us/index (serial descriptor
    processing in the Q7 ucode -- unusable), so the per-edge table
    gather happens on the host.  The degree-dot (sum_d eq*ek -> R=64, a
    0.6% sliver of total FLOPs) is fused into that gather, so the
    per-edge stream shipped to the device is w_ij bf16 -- 128 B/edge
    instead of the 1 KiB/edge of separate eq/ek streams.  All MLP
    matmul FLOPs (99.4% of the total) run on device.
  - The w stream packs subtile PAIRS into the 128 partitions (DMA
    engines stripe by partition/16, so a 64-partition transfer runs at
    half rate); stage-1 of mlp_w contracts rows [0,64) or [64,128)
    against a correspondingly duplicated w1.
  - Device per 500-edge subtile: 4 matmuls (mm1w 64-contract, mm1t,
    mm2w, mm2t) into a single 2-bank PSUM arena reused for stage-1 then
    stage-2 (their lifetimes don't overlap, so bufs=4 keeps 4 subtiles
    in flight in 8 banks); ONE merged 1000-col Act relu covers both
    branches' stage-1; p2w is staged to SBUF (DVE has one PSUM read
    port, so a two-PSUM-operand tensor_tensor is illegal -- the copy
    alternates DVE/Act to balance their load) and the final product
    runs on DVE, landing in a bf16 [128f, e] slab.
  - DMA: w+tt slabs on the SP queue (first phase chunked to cut
    startup), out quarter-slabs + packed weights on the Pool queue.
"""

import sys

if "/opt/trn_rl_repo" not in sys.path:
    sys.path.insert(0, "/opt/trn_rl_repo")

import ml_dtypes
import numpy as np

BF16 = ml_dtypes.bfloat16

# Problem dims (hardcoded per spec nn_HTR_7464653160731)
D, N, F, R, E, H = 4, 20000, 128, 64, 320000, 128
NCORES = 8

SUB = 500                 # edge subtile (matmul/PSUM granularity)
PHE = 10000               # edges per stream slab (one w/tt DMA each)
NSUBP = PHE // SUB        # subtiles per slab
EPAD = ((E // NCORES) + PHE - 1) // PHE * PHE   # 40000: no padding

_prog_cache = {}


def _build_program(epad, b2t_nonzero, b2w_nonzero, repeat=1,
                   b1t_nonzero=False, b1w_nonzero=False):
    import concourse.bacc as bacc
    import concourse.mybir as mybir
    import concourse.tile as tile

    f32 = mybir.dt.float32
    bf16 = mybir.dt.bfloat16
    AF = mybir.ActivationFunctionType
    ALU = mybir.AluOpType
    AX = mybir.AxisListType

    n_ph = epad // PHE
    nsp2 = NSUBP // 2
    b2_any = b2t_nonzero or b2w_nonzero

    nc = bacc.Bacc("TRN2", target_bir_lowering=False)

    # DRAM I/O.  wt holds per-edge w_ij rows packed two subtiles deep:
    # partition a*64+r of column (ph, u, c) is w_ij[edge (ph, 2u+a, c), r].
    wt = nc.dram_tensor("wt", [128, n_ph, nsp2, SUB], bf16,
                        kind="ExternalInput")
    tt = nc.dram_tensor("tt", [128, epad], bf16, kind="ExternalInput")
    mw1 = nc.dram_tensor("mw1", [128, H], bf16, kind="ExternalInput")
    mw2 = nc.dram_tensor("mw2", [128, F], bf16, kind="ExternalInput")
    mt1 = nc.dram_tensor("mt1", [128, H], bf16, kind="ExternalInput")
    mt2 = nc.dram_tensor("mt2", [128, F], bf16, kind="ExternalInput")
    bias = nc.dram_tensor("bias", [128, 4], f32, kind="ExternalInput")
    out = nc.dram_tensor("out", [128, epad], bf16, kind="ExternalOutput")

    any_bias = b1w_nonzero or b1t_nonzero or b2_any

    with tile.TileContext(nc) as tc:
        with tc.tile_pool(name="const", bufs=1) as cpool:
            mw1_sb = cpool.tile([128, H], bf16)
            mw2_sb = cpool.tile([128, F], bf16)
            mt1_sb = cpool.tile([128, H], bf16)
            mt2_sb = cpool.tile([128, F], bf16)
            consts = [(mw1_sb, mw1), (mw2_sb, mw2),
                      (mt1_sb, mt1), (mt2_sb, mt2)]
            if any_bias:
                bias_sb = cpool.tile([128, 4], f32)
                consts.append((bias_sb, bias))
            for sb_t, dr in consts:
                nc.gpsimd.dma_start(sb_t[:], dr[:])

            for _rep in range(repeat):
                with (
                    tc.tile_pool(name="wsl", bufs=2) as wsl,
                    tc.tile_pool(name="tsl", bufs=2) as tsl,
                    tc.tile_pool(name="osl", bufs=2) as osl,
                    tc.tile_pool(name="hbuf", bufs=6) as hbuf,
                    tc.tile_pool(name="swb", bufs=6) as swb,
                    tc.tile_pool(name="psm", bufs=4, space="PSUM") as psm,
                ):
                    for ph in range(n_ph):
                        sl_ph = slice(ph * PHE, (ph + 1) * PHE)
                        w_s = wsl.tile([128, nsp2, SUB], bf16, tag="wt")
                        tt_s = tsl.tile([128, PHE], bf16, tag="tt")
                        ot_s = osl.tile([128, PHE], bf16, tag="ot")
                        if ph == 0:
                            # chunk the first slab loads (1+3+3+3 pairs) so
                            # subtile 0 starts after a ~1-pair DMA
                            bounds = [0, 1, 4, 7, nsp2]
                            for k in range(4):
                                b0, b1 = bounds[k], bounds[k + 1]
                                nc.sync.dma_start(
                                    w_s[:, b0:b1, :],
                                    wt[:, ph, b0:b1, :])
                                nc.sync.dma_start(
                                    tt_s[:, b0 * 2 * SUB:b1 * 2 * SUB],
                                    tt[:, ph * PHE + b0 * 2 * SUB:
                                       ph * PHE + b1 * 2 * SUB])
                        else:
                            nc.sync.dma_start(w_s[:], wt[:, ph, :, :])
                            nc.sync.dma_start(tt_s[:], tt[:, sl_ph])
                        for j in range(NSUBP):
                            sl = slice(j * SUB, (j + 1) * SUB)
                            u, a = divmod(j, 2)
                            # one 2-bank PSUM arena per subtile, reused for
                            # stage-1 (p1w|p1t) then stage-2 (p2w|p2t): their
                            # lifetimes don't overlap, so bufs=4 gives 4
                            # subtiles in flight within 8 banks.
                            pq = psm.tile([128, 2, 512], f32, tag="pq")
                            nc.tensor.matmul(pq[:, 0, 0:SUB],
                                             mw1_sb[a * R:(a + 1) * R, :],
                                             w_s[a * R:(a + 1) * R, u, :],
                                             start=True, stop=True)
                            nc.tensor.matmul(pq[:, 1, 0:SUB], mt1_sb[:],
                                             tt_s[:, sl],
                                             start=True, stop=True)
                            h_t = hbuf.tile([128, 2, SUB], bf16, tag="h")
                            if b1w_nonzero or b1t_nonzero:
                                nc.scalar.activation(h_t[:, 0, :],
                                                     pq[:, 0, 0:SUB], AF.Relu,
                                                     bias=bias_sb[:, 0:1])
                                nc.scalar.activation(h_t[:, 1, :],
                                                     pq[:, 1, 0:SUB], AF.Relu,
                                                     bias=bias_sb[:, 1:2])
                            else:
                                # one 2*SUB-col act covers both relus
                                nc.scalar.activation(h_t[:],
                                                     pq[:, :, 0:SUB], AF.Relu)
                            # stage-2 overwrites the arena (WAR via h_t dep)
                            nc.tensor.matmul(pq[:, 0, 0:SUB], mw2_sb[:],
                                             h_t[:, 0, :],
                                             start=True, stop=True)
                            nc.tensor.matmul(pq[:, 1, 0:SUB], mt2_sb[:],
                                             h_t[:, 1, :],
                                             start=True, stop=True)
                            sw_t = swb.tile([128, SUB], f32, tag="sw")
                            if b2w_nonzero:
                                nc.scalar.activation(sw_t[:], pq[:, 0, 0:SUB],
                                                     AF.Identity,
                                                     bias=bias_sb[:, 2:3])
                            elif j % 5 in (1, 3):
                                # ~40% of staging copies on Act (balance probe)
                                nc.scalar.activation(sw_t[:], pq[:, 0, 0:SUB],
                                                     AF.Identity)
                            else:
                                nc.vector.tensor_copy(sw_t[:], pq[:, 0, 0:SUB])
                            if b2t_nonzero:
                                st_t = swb.tile([128, SUB], f32, tag="st")
                                nc.scalar.activation(st_t[:], pq[:, 1, 0:SUB],
                                                     AF.Identity,
                                                     bias=bias_sb[:, 3:4])
                                nc.vector.tensor_mul(ot_s[:, sl], st_t[:],
                                                     sw_t[:])
                            else:
                                nc.vector.tensor_mul(ot_s[:, sl],
                                                     pq[:, 1, 0:SUB], sw_t[:])
                            last_ph = ph == n_ph - 1
                            if last_ph and j == NSUBP - 3:
                                # last phase: drain subtiles 15-17 early so
                                # the final drain is only 2 subtiles long
                                c0, c1 = 3 * (PHE // 4), (NSUBP - 2) * SUB
                                nc.gpsimd.dma_start(
                                    out[:, ph * PHE + c0:ph * PHE + c1],
                                    ot_s[:, c0:c1])
                            elif last_ph and j == NSUBP - 1:
                                c0 = (NSUBP - 2) * SUB
                                nc.gpsimd.dma_start(
                                    out[:, ph * PHE + c0:(ph + 1) * PHE],
                                    ot_s[:, c0:])
                            elif (not last_ph or j < NSUBP - 3) and \
                                    j % (NSUBP // 4) == NSUBP // 4 - 1:
                                # quarter-slab output drains
                                k = j // (NSUBP // 4)
                                qc = PHE // 4
                                nc.gpsimd.dma_start(
                                    out[:, ph * PHE + k * qc:
                                        ph * PHE + (k + 1) * qc],
                                    ot_s[:, k * qc:(k + 1) * qc])


    nc.compile()
    return nc


def get_program(epad, b2t_nonzero, b2w_nonzero, b1t_nonzero=False,
                b1w_nonzero=False):
    key = (epad, b2t_nonzero, b2w_nonzero, b1t_nonzero, b1w_nonzero)
    if key not in _prog_cache:
        _prog_cache[key] = _build_program(
            epad, b2t_nonzero, b2w_nonzero,
            b1t_nonzero=b1t_nonzero, b1w_nonzero=b1w_nonzero)
    return _prog_cache[key]


def kernel(Xs, t_ij, edge_index, Wq, Wk, mw_w1, mw_b1, mw_w2, mw_b2,
           mt_w1, mt_b1, mt_w2, mt_b2):
    from concourse.bass_utils import run_bass_kernel_spmd

    Xs = np.asarray(Xs, np.float32)
    t_ij = np.asarray(t_ij, np.float32)
    edge_index = np.asarray(edge_index)

    esh = E // NCORES                      # edges per core
    epad = (esh + PHE - 1) // PHE * PHE

    nj = edge_index[0].astype(np.int64)
    ni = edge_index[1].astype(np.int64)

    # Node-level eq/ek tables (the replicated arrays from the sharding
    # hint), computed once in f32 on the host.
    Wq_ = np.asarray(Wq, np.float32)
    Wk_ = np.asarray(Wk, np.float32)
    eq_tab = np.einsum("dnf,fr->ndr", Xs, Wq_).astype(np.float32)
    ek_tab = np.einsum("dnf,dfr->ndr", Xs, Wk_).astype(np.float32)

    b1w_nonzero = bool(np.any(np.asarray(mw_b1) != 0))
    b1t_nonzero = bool(np.any(np.asarray(mt_b1) != 0))
    b2w_nonzero = bool(np.any(np.asarray(mw_b2) != 0))
    b2t_nonzero = bool(np.any(np.asarray(mt_b2) != 0))
    nc = get_program(epad, b2t_nonzero, b2w_nonzero, b1t_nonzero,
                     b1w_nonzero)

    bias_arr = np.zeros((128, 4), np.float32)
    bias_arr[:, 0] = np.asarray(mw_b1, np.float32)
    bias_arr[:, 1] = np.asarray(mt_b1, np.float32)
    bias_arr[:, 2] = np.asarray(mw_b2, np.float32)
    bias_arr[:, 3] = np.asarray(mt_b2, np.float32)

    com = {
        "mw1": np.ascontiguousarray(
            np.vstack([np.asarray(mw_w1)] * 2).astype(BF16)),
        "mw2": np.ascontiguousarray(np.asarray(mw_w2).astype(BF16)),
        "mt1": np.ascontiguousarray(np.asarray(mt_w1).astype(BF16)),
        "mt2": np.ascontiguousarray(np.asarray(mt_w2).astype(BF16)),
        "bias": bias_arr,
    }

    in_maps = []
    for g in range(NCORES):
        s0, s1 = g * esh, (g + 1) * esh
        # Fused host gather+degree-dot: w_ij[e, r] = sum_d eq[ni]*ek[nj].
        wij = np.einsum("edr,edr->er", eq_tab[ni[s0:s1]],
                        ek_tab[nj[s0:s1]])
        wpad = np.zeros((epad, R), np.float32)
        wpad[:esh] = wij
        # Pack pairs of SUB-edge subtiles into the 128 partitions:
        # wt[a*64+r, ph, u, c] = wij[edge (ph, 2u+a, c), r]
        n_ph = epad // PHE
        wv = wpad.reshape(n_ph, NSUBP // 2, 2, SUB, R)
        wv = wv.transpose(2, 4, 0, 1, 3).reshape(128, n_ph, NSUBP // 2, SUB)
        tpad = np.zeros((epad, F), np.float32)
        tpad[:esh] = t_ij[s0:s1]
        in_maps.append({
            **com,
            "wt": np.ascontiguousarray(wv.astype(BF16)),
            "tt": np.ascontiguousarray(tpad.T.astype(BF16)),
        })

    res = run_bass_kernel_spmd(nc, in_maps, list(range(NCORES))).results

    result = np.empty((E, F), np.float32)
    for g in range(NCORES):
        o = np.asarray(res[g]["out"]).astype(np.float32)  # [128, epad]
        result[g * esh:(g + 1) * esh] = o[:, :esh].T
    return result
